# revision 1
# baseline (speedup 1.0000x reference)
"""Deformable conv2d (ConvOffset2d) Trainium2 kernel.

Problem (hardcoded): x[8,64,128,128] f32, offset[8,72,128,128] f32,
weight[64,64,3,3] f32 -> out[8,64,128,128] f32.
KH=KW=3, stride 1, pad 1, CPG=16 (4 groups share offsets per 16 channels).

Data-parallel over batch: 1 image per NeuronCore, 8 cores. Per core:
  - image packed on host as u32 = (fp16 v[y,x] | fp16 v[y+1,x] << 16),
    doubled along x into (col, col+1) pairs so ONE ap_gather (d=2, u32)
    fetches all 4 bilinear corners x 16 channels per index.
  - image zero-padded (pad 10) and split into 2 overlapping row-halves to
    fit ap_gather's 32K-word addressing limit. OOB samples read zeros ==
    exact zero-padding semantics of the reference.
  - indices + fractional weights on DVE (magic-number floor); the 4 corner
    weights are (ay1,ay0)x(ax1,ax0) outer products via one 0-stride-AP
    multiply; the (+,-,-,+) sign pattern is folded into negated PE
    stationaries, which also makes exactly-integral offsets exact.
  - per-position weights replicated to 16 channel partitions by a one-hot
    matmul into PSUM; (gathered fp16) * (weights) on DVE; 4 matmuls per
    (set, corner) accumulate the (group,channel,tap) contraction in PSUM.
"""
import numpy as np

B, CIN, H, W = 8, 64, 128, 128
COUT = 64
G, CPG, K = 4, 16, 9
HO, WO = 128, 128
NPOS = HO * WO
PADV = 10                 # spatial zero-pad (supports |offset| < 9)
WP = W + 2 * PADV + 8     # padded row length 148 (cols -10..137)
ROWS_HALF = 84
QHALF = ROWS_HALF * WP    # 12432 u32 y-pairs per half
NSETS = 5
NC = 1024                 # positions per chunk (8 output rows)
NCHUNK = NPOS // NC
NSUB = 4
NCP = NC // NSUB          # 256
MAGIC = 12582912.0        # 1.5 * 2^23

_CACHE = {}


def _stream(t, c):
    g = c // 2
    if c % 2 == 0:
        return g, t, False
    return (g, 5 + t, False) if t < 4 else (g, 8, True)


def _build_nc(chunks=None):
    import concourse.bacc as bacc
    import concourse.bass as bass
    import concourse.mybir as mybir
    from concourse.tile import TileContext
    from concourse import library_config

    f32, f16, i16, u32 = (mybir.dt.float32, mybir.dt.float16,
                          mybir.dt.int16, mybir.dt.uint32)
    AL = mybir.AluOpType
    ts = bass.ts
    nc = bacc.Bacc("TRN2", target_bir_lowering=False, debug=False, num_devices=8)

    xpk = nc.dram_tensor("xpk", [2, 128, QHALF * 2], u32, kind="ExternalInput")
    offw = nc.dram_tensor("offw", [NSETS, 2, 128, NPOS // 16], f32, kind="ExternalInput")
    offp = nc.dram_tensor("offp", [2, NSETS * 8, NPOS], f32, kind="ExternalInput")
    bi = nc.dram_tensor("bi", [NSETS, 128, NPOS // 16], f32, kind="ExternalInput")
    onehot = nc.dram_tensor("onehot", [NSETS, NSETS * 8, 128], f16, kind="ExternalInput")
    wst = nc.dram_tensor("wst", [NSETS * 4, 128, COUT], f16, kind="ExternalInput")
    out = nc.dram_tensor("out", [COUT, NPOS], f32, kind="ExternalOutput")
    dbg_w4 = nc.dram_tensor("dbg_w4", [NSETS * 8, NC * 4], f16, kind="ExternalOutput")
    dbg_g = nc.dram_tensor("dbg_g", [128, NC * 2], u32, kind="ExternalOutput")
    dbg_m = nc.dram_tensor("dbg_m", [128, NCP * 4], f16, kind="ExternalOutput")


    with TileContext(nc) as tc:
        with tc.tile_pool(name="res", bufs=1) as res, \
             tc.tile_pool(name="img", bufs=1) as imgp, \
             tc.tile_pool(name="wk", bufs=2) as wk, \
             tc.tile_pool(name="w1", bufs=1) as w1, \
             tc.tile_pool(name="ps", bufs=2, space="PSUM") as ps, \
             tc.tile_pool(name="psmm", bufs=2, space="PSUM") as psmm:

            nc.gpsimd.load_library(library_config.ap_gather)

            oh_t = res.tile([NSETS * 8, NSETS, 128], f16)
            for t in range(NSETS):
                nc.sync.dma_start(out=oh_t[:, t, :], in_=onehot[t])
            wst_t = res.tile([128, NSETS * 4, COUT], f16)
            for i in range(NSETS * 4):
                nc.sync.dma_start(out=wst_t[:, i, :], in_=wst[i])
            idx_t = res.tile([128, NSETS, NPOS // 16], i16)

            # ---- index pipeline (wrapped layout), scoped pool ----
            with tc.tile_pool(name="ix", bufs=1) as ix:
                NQ = NPOS // 16
                hc = NQ // 2
                for t in range(NSETS):
                    for hh in range(2):
                        cs = slice(hh * hc, (hh + 1) * hc)
                        dyw = ix.tile([128, hc], f32, tag="dA")
                        nc.sync.dma_start(out=dyw[:], in_=offw[t, 0, :, cs])
                        dxw = ix.tile([128, hc], f32, tag="dB")
                        nc.sync.dma_start(out=dxw[:], in_=offw[t, 1, :, cs])
                        bi_t = ix.tile([128, hc], f32, tag="bi")
                        nc.sync.dma_start(out=bi_t[:], in_=bi[t, :, cs])

                        ty = ix.tile([128, hc], f32, tag="tmp")
                        nc.vector.tensor_scalar(ty[:], dyw[:], 0.5, MAGIC,
                                                AL.subtract, AL.add)
                        y0 = ix.tile([128, hc], f32, tag="dA")
                        nc.vector.tensor_scalar(y0[:], ty[:], MAGIC, None,
                                                AL.subtract)
                        tx = ix.tile([128, hc], f32, tag="tmp")
                        nc.vector.tensor_scalar(tx[:], dxw[:], 0.5, MAGIC,
                                                AL.subtract, AL.add)
                        x0 = ix.tile([128, hc], f32, tag="dB")
                        nc.vector.tensor_scalar(x0[:], tx[:], MAGIC, None,
                                                AL.subtract)
                        rel = ix.tile([128, hc], f32, tag="rel")
                        nc.vector.scalar_tensor_tensor(rel[:], y0[:], float(WP),
                                                       x0[:], AL.mult, AL.add)
                        nc.vector.scalar_tensor_tensor(
                            idx_t[:, t, cs], rel[:], float(-64 * WP) * hh,
                            bi_t[:], AL.add, AL.add)

            img_t = imgp.tile([128, QHALF * 2], u32)
            nc.sync.dma_start(out=img_t[:], in_=xpk[0])
            imgv = img_t[:].rearrange("p (n d) -> p n d", d=2)

            # ---- main loop over 16 position chunks ----
            for ch in (range(NCHUNK) if chunks is None else chunks):
                if ch >= NCHUNK // 2 and (chunks is not None or ch == NCHUNK // 2):
                    img_t = imgp.tile([128, QHALF * 2], u32)
                    nc.sync.dma_start(out=img_t[:], in_=xpk[1])
                    imgv = img_t[:].rearrange("p (n d) -> p n d", d=2)

                dyp = wk.tile([NSETS * 8, NC], f32, tag="dyp")
                nc.sync.dma_start(out=dyp[:], in_=offp[0, :, ts(ch, NC)])
                dxp = wk.tile([NSETS * 8, NC], f32, tag="dxp")
                nc.sync.dma_start(out=dxp[:], in_=offp[1, :, ts(ch, NC)])

                ayi = w1.tile([NSETS * 8, NC, 2], f32, tag="ayi")
                axi = w1.tile([NSETS * 8, NC, 2], f32, tag="axi")
                for (dp, wi) in ((dyp, ayi), (dxp, axi)):
                    tt = w1.tile([NSETS * 8, NC], f32, tag="tt")
                    nc.vector.tensor_scalar(tt[:], dp[:], 0.5, MAGIC,
                                            AL.subtract, AL.add)
                    nc.vector.scalar_tensor_tensor(wi[:, :, 1], tt[:], MAGIC, dp[:],
                                                   AL.subtract, AL.subtract)
                    nc.vector.tensor_scalar(wi[:, :, 0], wi[:, :, 1], 1.0, None,
                                            AL.add)
                # W4[s,n,j], j=(cy,cx) in 00,10,01,11; in0=(ay1,ay0|ay1,ay0),
                # in1=(ax1,ax1|ax0,ax0) via 0-stride APs
                w4 = wk.tile([NSETS * 8, NC, 2, 2], f16, tag="w4")
                a0 = ayi[:]
                a1 = axi[:]
                in0 = bass.AP(a0.tensor, a0.offset, [a0.ap[0], [2, NC], [0, 2], [1, 2]])
                in1 = bass.AP(a1.tensor, a1.offset, [a1.ap[0], [2, NC], [1, 2], [0, 2]])
                nc.vector.tensor_tensor(w4[:], in0, in1, AL.mult)
                if ch == 0:
                    nc.sync.dma_start(out=dbg_w4[:], in_=w4[:].rearrange("p n x y -> p (n x y)"))
                w4f = w4[:].rearrange("p n x y -> p (n x y)")

                mm = psmm.tile([COUT, NC], f32)
                nc.vector.memset(mm[:], 0.0)
                for t in range(NSETS):
                    gout = wk.tile([128, NC, 2], u32, tag="gout")
                    nc.gpsimd.ap_gather(
                        gout[:], imgv, idx_t[:, t, ts(ch, NC // 16)],
                        channels=128, num_elems=QHALF, d=2, num_idxs=NC)
                    if ch == 0 and t == 0:
                        nc.sync.dma_start(out=dbg_g[:], in_=gout[:].rearrange("p n d -> p (n d)"))
                    gv = gout[:].rearrange("p n d -> p (n d)").bitcast(f16) \
                                .rearrange("p (n j) -> p n j", j=4)
                    for sc in range(NSUB):
                        w4p = ps.tile([128, NCP * 4], f32, tag="w4p")
                        for h in range(2):
                            nc.tensor.matmul(
                                w4p[:, ts(h, NCP * 2)], oh_t[:, t, :],
                                w4f[:, sc * NCP * 4 + h * NCP * 2:
                                    sc * NCP * 4 + (h + 1) * NCP * 2],
                                start=True, stop=True)
                        m = wk.tile([128, NCP, 4], f16, tag="m")
                        nc.vector.tensor_tensor(
                            m[:], gv[:, ts(sc, NCP), :],
                            w4p[:].rearrange("p (n j) -> p n j", j=4), AL.mult)
                        if ch == 0 and t == 0 and sc == 0:
                            nc.sync.dma_start(out=dbg_m[:], in_=m[:].rearrange("p n j -> p (n j)"))
                        for j in range(4):
                            nc.tensor.matmul(
                                mm[:, ts(sc, NCP)], wst_t[:, 4 * t + j, :],
                                m[:, :, j],
                                start=False,
                                stop=(t == NSETS - 1 and j == 3))
                ob = wk.tile([COUT, NC], f32, tag="ob")
                for sc in range(NSUB):
                    nc.scalar.copy(ob[:, ts(sc, NCP)], mm[:, ts(sc, NCP)])
                nc.sync.dma_start(out=out[:, ts(ch, NC)], in_=ob[:])

    nc.compile()
    return nc


def _host_pack(x, offset, weight):
    xf = np.asarray(x, np.float32)
    off = np.asarray(offset, np.float32)
    wt = np.asarray(weight, np.float32)
    assert np.abs(off).max() < 9.0, "offset exceeds supported pad range"

    RT = H + 2 * PADV + 9
    xpad = np.zeros((B, CIN, RT, WP), np.float16)
    xpad[:, :, PADV:PADV + H, PADV:PADV + W] = xf.astype(np.float16)

    xpk = np.zeros((B, 2, 128, QHALF * 2), np.uint32)
    for h, rb in ((0, 0), (1, 64)):
        rows = xpad[:, :, rb:rb + ROWS_HALF, :]
        rows1 = xpad[:, :, rb + 1:rb + 1 + ROWS_HALF, :]
        pair = (rows1.view(np.uint16).astype(np.uint32) << 16) | \
               rows.view(np.uint16).astype(np.uint32)
        pairq = pair.reshape(B, CIN, QHALF)
        dbl = np.zeros((B, CIN, QHALF, 2), np.uint32)
        dbl[:, :, :, 0] = pairq
        dbl[:, :, :-1, 1] = pairq[:, :, 1:]
        for c in range(8):
            g = c // 2
            xpk[:, h, 16 * c:16 * c + 16, :] = dbl[:, 16 * g:16 * g + 16].reshape(
                B, 16, QHALF * 2)

    offr = off.reshape(B, G, K, 2, NPOS)
    offw = np.zeros((B, NSETS, 2, 128, NPOS // 16), np.float32)
    offp = np.zeros((B, 2, NSETS * 8, NPOS), np.float32)
    bi = np.zeros((NSETS, 128, NPOS // 16), np.float32)
    wstk = np.zeros((NSETS * 4, 128, COUT), np.float16)
    p = np.arange(NPOS)
    ho, wo = p >> 7, p & 127
    sgn = (1.0, -1.0, -1.0, 1.0)
    wr = wt.reshape(COUT, G, CPG, K)
    for t in range(NSETS):
        for c in range(8):
            g, k, is_pad = _stream(t, c)
            dy, dx = offr[:, g, k, 0], offr[:, g, k, 1]
            offw[:, t, 0, 16 * c:16 * c + 16, :] = dy.reshape(
                B, NPOS // 16, 16).transpose(0, 2, 1)
            offw[:, t, 1, 16 * c:16 * c + 16, :] = dx.reshape(
                B, NPOS // 16, 16).transpose(0, 2, 1)
            offp[:, 0, 8 * t + c, :] = dy
            offp[:, 1, 8 * t + c, :] = dx
            ky, kx = k // 3, k % 3
            biv = ((ho + (ky - 1) + PADV) * WP + (wo + (kx - 1) + PADV)).astype(
                np.float32)
            bi[t, 16 * c:16 * c + 16, :] = biv.reshape(NPOS // 16, 16).T
            if not is_pad:
                for j in range(4):
                    wstk[4 * t + j, 16 * c:16 * c + 16, :] = \
                        (sgn[j] * wr[:, g, :, k]).T.astype(np.float16)

    onehot = np.zeros((NSETS, NSETS * 8, 128), np.float16)
    for t in range(NSETS):
        for c in range(8):
            onehot[t, 8 * t + c, 16 * c:16 * c + 16] = 1.0
    return xpk, offw, offp, bi, onehot, wstk


def kernel(x, offset, weight):
    if "nc" not in _CACHE:
        _CACHE["nc"] = _build_nc()
    nc = _CACHE["nc"]
    from concourse.bass_utils import run_bass_kernel_spmd

    xpk, offw, offp, bi, onehot, wstk = _host_pack(x, offset, weight)
    in_maps = [dict(xpk=xpk[b], offw=offw[b], offp=offp[b],
                    bi=bi, onehot=onehot, wst=wstk) for b in range(B)]
    res = run_bass_kernel_spmd(nc, in_maps, core_ids=list(range(B)))
    out = np.stack([res.results[b]["out"] for b in range(B)], axis=0)
    return out.reshape(B, COUT, HO, WO).astype(np.float32)

